# revision 2
# baseline (speedup 1.0000x reference)
"""Trainium2 Bass kernel for nn_DeepBSpline (per-channel uniform-knot linear
B-spline activation with linear extrapolation).

Approach: the whole op (clamp + bin + two gathers + lerp + extrapolation) is,
per channel, a single continuous piecewise-linear function of x with at most
50 pieces whose kinks sit at the (compile-time-known) knot grid.  At call
time the host compresses the coefficient table into its minimal relu basis

    f_c(x) = alpha_c + beta_c * x + sum_j D_cj * relu(x - b_cj)

keeping only kinks with a non-negligible slope change.  The device kernel is
then gather-free: one fused multiply-add (VectorE tensor_scalar) plus, per
kept kink, one biased Relu (ScalarE activation) and one fused multiply-add
(VectorE scalar_tensor_tensor), with per-partition scalars carrying the
per-channel constants.  The number of kept kinks T is the only thing baked
into the program; all values flow through a small "consts" input tensor, so
the compiled NEFF is reusable across coefficient values with the same T.

Sharding: data-parallel over the batch dim — 8 cores x 2 batches each; each
core's (2, 64, 256, 256) slab is viewed as [128 partitions, 65536] with
partition p = b*64 + c, so per-channel constants become per-partition scalars.
"""

import os
import sys

import numpy as np

for _p in ("/opt/trn_rl_repo", "/root/.axon_site", "/root/.axon_site/_ro/trn_rl_repo",
           "/root/.axon_site/_ro/pypackages"):
    if os.path.isdir(_p) and _p not in sys.path:
        sys.path.append(_p)

GRID = 0.16
SIZE = 51
HALF = SIZE // 2
C = 64
N_BATCH = 16
HW = 256 * 256
N_CORES = 8
P = 128                      # partitions = 2 batches x 64 channels
BATCH_PER_CORE = N_BATCH // N_CORES
FREE = BATCH_PER_CORE * C * HW // P   # 65536 free-dim elements per partition
F_TILE = 4096


def _build_pwl(coefficients_vect, tol_rel=1e-4):
    """Compress the spline table to relu-basis PWL coefficients (float64).

    Returns alpha[C], beta[C], terms (per channel list of (kink_x, slope_delta)),
    and the max term count across channels.
    """
    cv = np.asarray(coefficients_vect, np.float64).reshape(C, SIZE)
    slopes_x = np.diff(cv, axis=1) / GRID          # (C, 50) per-bin slopes
    dd = np.diff(slopes_x, axis=1)                 # (C, 49) slope changes at knots 1..49
    scale = np.abs(dd).max() + 1e-30
    keep = np.abs(dd) > tol_rel * scale
    alpha = np.empty(C)
    beta = np.empty(C)
    terms = []
    max_terms = 0
    for c in range(C):
        ks = [0] + list(np.nonzero(keep[c])[0] + 1) + [SIZE - 1]
        # refit chords so the PWL interpolates the exact table values at the
        # kept kinks and both endpoints
        k0, k1 = ks[0], ks[1]
        s0 = (cv[c, k1] - cv[c, k0]) / ((k1 - k0) * GRID)
        beta[c] = s0
        alpha[c] = cv[c, k0] - (k0 - HALF) * GRID * s0
        t = []
        prev_s = s0
        for i in range(1, len(ks) - 1):
            ka, kb = ks[i], ks[i + 1]
            s = (cv[c, kb] - cv[c, ka]) / ((kb - ka) * GRID)
            t.append(((ka - HALF) * GRID, s - prev_s))
            prev_s = s
        terms.append(t)
        max_terms = max(max_terms, len(t))
    return alpha, beta, terms, max_terms


def _consts_array(alpha, beta, terms, T):
    """[P, 2+2T] float32: per partition (b*64+c): alpha, beta, (-b_j, D_j)*T."""
    K = 2 + 2 * T
    a = np.zeros((C, K), np.float32)
    a[:, 0] = np.asarray(alpha, np.float32)
    a[:, 1] = np.asarray(beta, np.float32)
    for c in range(C):
        for j, (b, d) in enumerate(terms[c]):
            a[c, 2 + 2 * j] = np.float32(-b)
            a[c, 3 + 2 * j] = np.float32(d)
    return np.tile(a, (P // C, 1)).astype(np.float32)


def _build_bass(T, free=FREE, f_tile=F_TILE):
    """Emit + compile the Bass/Tile program for term count T."""
    from contextlib import ExitStack

    import concourse.bass as bass
    import concourse.tile as tile
    from concourse import bacc, mybir

    nc = bacc.Bacc("TRN2", target_bir_lowering=False, debug=False,
                   num_devices=N_CORES)
    f32 = mybir.dt.float32
    x_d = nc.dram_tensor("x", [P, free], f32, kind="ExternalInput")
    c_d = nc.dram_tensor("consts", [P, 2 + 2 * T], f32, kind="ExternalInput")
    o_d = nc.dram_tensor("out", [P, free], f32, kind="ExternalOutput")
    n_tiles = free // f_tile
    assert n_tiles * f_tile == free

    mul = mybir.AluOpType.mult
    add = mybir.AluOpType.add
    relu = mybir.ActivationFunctionType.Relu

    with tile.TileContext(nc) as tc, ExitStack() as ctx:
        cpool = ctx.enter_context(tc.tile_pool(name="cpool", bufs=1))
        ct = cpool.tile([P, 2 + 2 * T], f32)
        nc.sync.dma_start(ct[:], c_d.ap())

        xin = ctx.enter_context(tc.tile_pool(name="xin", bufs=3))
        fp = ctx.enter_context(tc.tile_pool(name="fp", bufs=2))
        rp = ctx.enter_context(tc.tile_pool(name="rp", bufs=2))
        op = ctx.enter_context(tc.tile_pool(name="op", bufs=3))

        for i in range(n_tiles):
            xt = xin.tile([P, f_tile], f32)
            nc.sync.dma_start(xt[:], x_d.ap()[:, bass.ts(i, f_tile)])

            acc = fp.tile([P, f_tile], f32)
            nc.vector.tensor_scalar(acc[:], xt[:], ct[:, 1:2], ct[:, 0:1], mul, add)

            for j in range(T):
                rt = rp.tile([P, f_tile], f32)
                nc.scalar.activation(rt[:], xt[:], relu,
                                     bias=ct[:, 2 + 2 * j:3 + 2 * j])
                ot = op.tile([P, f_tile], f32)
                nc.vector.scalar_tensor_tensor(ot[:], rt[:],
                                               ct[:, 3 + 2 * j:4 + 2 * j],
                                               acc[:], mul, add)
                acc = ot

            nc.sync.dma_start(o_d.ap()[:, bass.ts(i, f_tile)], acc[:])

    nc.compile()
    return nc


_NC_CACHE = {}


def _get_nc(T):
    if T not in _NC_CACHE:
        _NC_CACHE[T] = _build_bass(T)
    return _NC_CACHE[T]


def kernel(x, coefficients_vect, size):
    assert int(size) == SIZE
    x = np.ascontiguousarray(np.asarray(x, np.float32))
    assert x.shape == (N_BATCH, C, 256, 256)
    cv = np.asarray(coefficients_vect, np.float32)

    alpha, beta, terms, T = _build_pwl(cv)
    T = max(T, 1)
    consts = _consts_array(alpha, beta, terms, T)

    from concourse.bass_utils import run_bass_kernel_spmd

    nc = _get_nc(T)
    in_maps = [
        {"x": x[i * BATCH_PER_CORE:(i + 1) * BATCH_PER_CORE].reshape(P, FREE),
         "consts": consts}
        for i in range(N_CORES)
    ]
    res = run_bass_kernel_spmd(nc, in_maps, list(range(N_CORES))).results
    out = np.concatenate(
        [r["out"].reshape(BATCH_PER_CORE, C, 256, 256) for r in res], axis=0
    )
    return out


# revision 4
# speedup vs baseline: 1.0504x; 1.0504x over previous
"""Trainium2 Bass kernel for nn_DeepBSpline (per-channel uniform-knot linear
B-spline activation with linear extrapolation).

Approach: the whole op (clamp + bin + two gathers + lerp + extrapolation) is,
per channel, a single continuous piecewise-linear function of x with at most
50 pieces whose kinks sit at the (compile-time-known) knot grid.  At call
time the host compresses the coefficient table into its minimal relu basis

    f_c(x) = alpha_c + beta_c * x + sum_j D_cj * relu(x - b_cj)

keeping only kinks with a non-negligible slope change.  The device kernel is
then gather-free: one fused multiply-add (VectorE tensor_scalar) plus, per
kept kink, one biased Relu (ScalarE activation) and one fused multiply-add
(VectorE scalar_tensor_tensor), with per-partition scalars carrying the
per-channel constants.  The number of kept kinks T is the only thing baked
into the program; all values flow through a small "consts" input tensor, so
the compiled NEFF is reusable across coefficient values with the same T.

Sharding: data-parallel over the batch dim — 8 cores x 2 batches each; each
core's (2, 64, 256, 256) slab is viewed as [128 partitions, 65536] with
partition p = b*64 + c, so per-channel constants become per-partition scalars.
"""

import os
import sys

import numpy as np

for _p in ("/opt/trn_rl_repo", "/root/.axon_site", "/root/.axon_site/_ro/trn_rl_repo",
           "/root/.axon_site/_ro/pypackages"):
    if os.path.isdir(_p) and _p not in sys.path:
        sys.path.append(_p)

GRID = 0.16
SIZE = 51
HALF = SIZE // 2
C = 64
N_BATCH = 16
HW = 256 * 256
N_CORES = 8
P = 128                      # partitions = 2 batches x 64 channels
BATCH_PER_CORE = N_BATCH // N_CORES
FREE = BATCH_PER_CORE * C * HW // P   # 65536 free-dim elements per partition
F_TILE = 2048


def _build_pwl(coefficients_vect, tol_rel=1e-4):
    """Compress the spline table to relu-basis PWL coefficients (float64).

    Returns alpha[C], beta[C], terms (per channel list of (kink_x, slope_delta)),
    and the max term count across channels.
    """
    cv = np.asarray(coefficients_vect, np.float64).reshape(C, SIZE)
    slopes_x = np.diff(cv, axis=1) / GRID          # (C, 50) per-bin slopes
    dd = np.diff(slopes_x, axis=1)                 # (C, 49) slope changes at knots 1..49
    scale = np.abs(dd).max() + 1e-30
    keep = np.abs(dd) > tol_rel * scale
    alpha = np.empty(C)
    beta = np.empty(C)
    terms = []
    max_terms = 0
    for c in range(C):
        ks = [0] + list(np.nonzero(keep[c])[0] + 1) + [SIZE - 1]
        # refit chords so the PWL interpolates the exact table values at the
        # kept kinks and both endpoints
        k0, k1 = ks[0], ks[1]
        s0 = (cv[c, k1] - cv[c, k0]) / ((k1 - k0) * GRID)
        beta[c] = s0
        alpha[c] = cv[c, k0] - (k0 - HALF) * GRID * s0
        t = []
        prev_s = s0
        for i in range(1, len(ks) - 1):
            ka, kb = ks[i], ks[i + 1]
            s = (cv[c, kb] - cv[c, ka]) / ((kb - ka) * GRID)
            t.append(((ka - HALF) * GRID, s - prev_s))
            prev_s = s
        terms.append(t)
        max_terms = max(max_terms, len(t))
    return alpha, beta, terms, max_terms


def _consts_array(alpha, beta, terms, T):
    """[P, 2+2T] float32: per partition (b*64+c): alpha, beta, (-b_j, D_j)*T."""
    K = 2 + 2 * T
    a = np.zeros((C, K), np.float32)
    a[:, 0] = np.asarray(alpha, np.float32)
    a[:, 1] = np.asarray(beta, np.float32)
    for c in range(C):
        for j, (b, d) in enumerate(terms[c]):
            a[c, 2 + 2 * j] = np.float32(-b)
            a[c, 3 + 2 * j] = np.float32(d)
    return np.tile(a, (P // C, 1)).astype(np.float32)


def _build_bass(T, free=FREE, f_tile=F_TILE):
    """Emit + compile the Bass/Tile program for term count T."""
    from contextlib import ExitStack

    import concourse.bass as bass
    import concourse.tile as tile
    from concourse import bacc, mybir

    nc = bacc.Bacc("TRN2", target_bir_lowering=False, debug=False,
                   num_devices=N_CORES)
    f32 = mybir.dt.float32
    x_d = nc.dram_tensor("x", [P, free], f32, kind="ExternalInput")
    c_d = nc.dram_tensor("consts", [P, 2 + 2 * T], f32, kind="ExternalInput")
    o_d = nc.dram_tensor("out", [P, free], f32, kind="ExternalOutput")
    n_tiles = free // f_tile
    assert n_tiles * f_tile == free

    mul = mybir.AluOpType.mult
    add = mybir.AluOpType.add
    relu = mybir.ActivationFunctionType.Relu

    with tile.TileContext(nc) as tc, ExitStack() as ctx:
        cpool = ctx.enter_context(tc.tile_pool(name="cpool", bufs=1))
        ct = cpool.tile([P, 2 + 2 * T], f32)
        nc.sync.dma_start(ct[:], c_d.ap())

        xin = ctx.enter_context(tc.tile_pool(name="xin", bufs=6))
        fp = ctx.enter_context(tc.tile_pool(name="fp", bufs=4))
        rp = ctx.enter_context(tc.tile_pool(name="rp", bufs=4))
        op = ctx.enter_context(tc.tile_pool(name="op", bufs=6))

        for i in range(n_tiles):
            xt = xin.tile([P, f_tile], f32)
            # loads issued from the ACT sequencer (qACT HWDGE queue) so that
            # stores (qSP via nc.sync) never head-of-line-block the loads
            nc.scalar.dma_start(xt[:], x_d.ap()[:, bass.ts(i, f_tile)])

            acc = fp.tile([P, f_tile], f32)
            nc.vector.tensor_scalar(acc[:], xt[:], ct[:, 1:2], ct[:, 0:1], mul, add)

            for j in range(T):
                rt = rp.tile([P, f_tile], f32)
                nc.scalar.activation(rt[:], xt[:], relu,
                                     bias=ct[:, 2 + 2 * j:3 + 2 * j])
                ot = op.tile([P, f_tile], f32)
                nc.vector.scalar_tensor_tensor(ot[:], rt[:],
                                               ct[:, 3 + 2 * j:4 + 2 * j],
                                               acc[:], mul, add)
                acc = ot

            nc.sync.dma_start(o_d.ap()[:, bass.ts(i, f_tile)], acc[:])

    nc.compile()
    return nc


_NC_CACHE = {}


def _get_nc(T):
    if T not in _NC_CACHE:
        _NC_CACHE[T] = _build_bass(T)
    return _NC_CACHE[T]


def kernel(x, coefficients_vect, size):
    assert int(size) == SIZE
    x = np.ascontiguousarray(np.asarray(x, np.float32))
    assert x.shape == (N_BATCH, C, 256, 256)
    cv = np.asarray(coefficients_vect, np.float32)

    alpha, beta, terms, T = _build_pwl(cv)
    T = max(T, 1)
    consts = _consts_array(alpha, beta, terms, T)

    from concourse.bass_utils import run_bass_kernel_spmd

    nc = _get_nc(T)
    in_maps = [
        {"x": x[i * BATCH_PER_CORE:(i + 1) * BATCH_PER_CORE].reshape(P, FREE),
         "consts": consts}
        for i in range(N_CORES)
    ]
    res = run_bass_kernel_spmd(nc, in_maps, list(range(N_CORES))).results
    out = np.concatenate(
        [r["out"].reshape(BATCH_PER_CORE, C, 256, 256) for r in res], axis=0
    )
    return out


# revision 6
# speedup vs baseline: 1.1631x; 1.1073x over previous
"""Trainium2 Bass kernel for nn_DeepBSpline (per-channel uniform-knot linear
B-spline activation with linear extrapolation).

Approach: the whole op (clamp + bin + two gathers + lerp + extrapolation) is,
per channel, a single continuous piecewise-linear function of x with at most
50 pieces whose kinks sit at the (compile-time-known) knot grid.  At call
time the host compresses the coefficient table into its minimal relu basis

    f_c(x) = alpha_c + beta_c * x + sum_j D_cj * relu(x - b_cj)

keeping only kinks with a non-negligible slope change.  The device kernel is
then gather-free: one fused multiply-add (VectorE tensor_scalar) plus, per
kept kink, one biased Relu (ScalarE activation) and one fused multiply-add
(VectorE scalar_tensor_tensor), with per-partition scalars carrying the
per-channel constants.  The number of kept kinks T is the only thing baked
into the program; all values flow through a small "consts" input tensor, so
the compiled NEFF is reusable across coefficient values with the same T.

Sharding: data-parallel over the batch dim — 8 cores x 2 batches each; each
core's (2, 64, 256, 256) slab is viewed as [128 partitions, 65536] with
partition p = b*64 + c, so per-channel constants become per-partition scalars.
"""

import os
import sys

import numpy as np

for _p in ("/opt/trn_rl_repo", "/root/.axon_site", "/root/.axon_site/_ro/trn_rl_repo",
           "/root/.axon_site/_ro/pypackages"):
    if os.path.isdir(_p) and _p not in sys.path:
        sys.path.append(_p)

GRID = 0.16
SIZE = 51
HALF = SIZE // 2
C = 64
N_BATCH = 16
HW = 256 * 256
N_CORES = 8
P = 128                      # partitions = 2 batches x 64 channels
BATCH_PER_CORE = N_BATCH // N_CORES
FREE = BATCH_PER_CORE * C * HW // P   # 65536 free-dim elements per partition
F_TILE = 4096


def _build_pwl(coefficients_vect, tol_rel=1e-4):
    """Compress the spline table to relu-basis PWL coefficients (float64).

    Returns alpha[C], beta[C], terms (per channel list of (kink_x, slope_delta)),
    and the max term count across channels.
    """
    cv = np.asarray(coefficients_vect, np.float64).reshape(C, SIZE)
    slopes_x = np.diff(cv, axis=1) / GRID          # (C, 50) per-bin slopes
    dd = np.diff(slopes_x, axis=1)                 # (C, 49) slope changes at knots 1..49
    scale = np.abs(dd).max() + 1e-30
    keep = np.abs(dd) > tol_rel * scale
    alpha = np.empty(C)
    beta = np.empty(C)
    terms = []
    max_terms = 0
    for c in range(C):
        ks = [0] + list(np.nonzero(keep[c])[0] + 1) + [SIZE - 1]
        # refit chords so the PWL interpolates the exact table values at the
        # kept kinks and both endpoints
        k0, k1 = ks[0], ks[1]
        s0 = (cv[c, k1] - cv[c, k0]) / ((k1 - k0) * GRID)
        beta[c] = s0
        alpha[c] = cv[c, k0] - (k0 - HALF) * GRID * s0
        t = []
        prev_s = s0
        for i in range(1, len(ks) - 1):
            ka, kb = ks[i], ks[i + 1]
            s = (cv[c, kb] - cv[c, ka]) / ((kb - ka) * GRID)
            t.append(((ka - HALF) * GRID, s - prev_s))
            prev_s = s
        terms.append(t)
        max_terms = max(max_terms, len(t))
    return alpha, beta, terms, max_terms


def _consts_array(alpha, beta, terms, T):
    """[P, 2+2T] float32: per partition (b*64+c): alpha, beta, (-b_j, D_j)*T."""
    K = 2 + 2 * T
    a = np.zeros((C, K), np.float32)
    a[:, 0] = np.asarray(alpha, np.float32)
    a[:, 1] = np.asarray(beta, np.float32)
    for c in range(C):
        for j, (b, d) in enumerate(terms[c]):
            a[c, 2 + 2 * j] = np.float32(-b)
            a[c, 3 + 2 * j] = np.float32(d)
    return np.tile(a, (P // C, 1)).astype(np.float32)


def _build_bass(T, free=FREE, f_tile=F_TILE):
    """Emit + compile the Bass/Tile program for term count T."""
    from contextlib import ExitStack

    import concourse.bass as bass
    import concourse.tile as tile
    from concourse import bacc, mybir

    nc = bacc.Bacc("TRN2", target_bir_lowering=False, debug=False,
                   num_devices=N_CORES)
    f32 = mybir.dt.float32
    x_d = nc.dram_tensor("x", [P, free], f32, kind="ExternalInput")
    c_d = nc.dram_tensor("consts", [P, 2 + 2 * T], f32, kind="ExternalInput")
    o_d = nc.dram_tensor("out", [P, free], f32, kind="ExternalOutput")
    n_tiles = free // f_tile
    assert n_tiles * f_tile == free

    mul = mybir.AluOpType.mult
    add = mybir.AluOpType.add
    relu = mybir.ActivationFunctionType.Relu

    with tile.TileContext(nc) as tc, ExitStack() as ctx:
        cpool = ctx.enter_context(tc.tile_pool(name="cpool", bufs=1))
        ct = cpool.tile([P, 2 + 2 * T], f32)
        nc.sync.dma_start(ct[:], c_d.ap())

        xin = ctx.enter_context(tc.tile_pool(name="xin", bufs=4))
        fp = ctx.enter_context(tc.tile_pool(name="fp", bufs=2))
        rp = ctx.enter_context(tc.tile_pool(name="rp", bufs=2))
        op = ctx.enter_context(tc.tile_pool(name="op", bufs=3))

        for i in range(n_tiles):
            xt = xin.tile([P, f_tile], f32)
            # loads issued from the ACT sequencer (qACT HWDGE queue) so that
            # stores (qSP via nc.sync) never head-of-line-block the loads
            nc.scalar.dma_start(xt[:], x_d.ap()[:, bass.ts(i, f_tile)])

            acc = fp.tile([P, f_tile], f32)
            nc.vector.tensor_scalar(acc[:], xt[:], ct[:, 1:2], ct[:, 0:1], mul, add)

            for j in range(T):
                rt = rp.tile([P, f_tile], f32)
                nc.scalar.activation(rt[:], xt[:], relu,
                                     bias=ct[:, 2 + 2 * j:3 + 2 * j])
                ot = op.tile([P, f_tile], f32)
                nc.vector.scalar_tensor_tensor(ot[:], rt[:],
                                               ct[:, 3 + 2 * j:4 + 2 * j],
                                               acc[:], mul, add)
                acc = ot

            nc.sync.dma_start(o_d.ap()[:, bass.ts(i, f_tile)], acc[:])

    nc.compile()
    return nc


_NC_CACHE = {}


def _get_nc(T):
    if T not in _NC_CACHE:
        _NC_CACHE[T] = _build_bass(T)
    return _NC_CACHE[T]


def kernel(x, coefficients_vect, size):
    assert int(size) == SIZE
    x = np.ascontiguousarray(np.asarray(x, np.float32))
    assert x.shape == (N_BATCH, C, 256, 256)
    cv = np.asarray(coefficients_vect, np.float32)

    alpha, beta, terms, T = _build_pwl(cv)
    T = max(T, 1)
    consts = _consts_array(alpha, beta, terms, T)

    from concourse.bass_utils import run_bass_kernel_spmd

    nc = _get_nc(T)
    in_maps = [
        {"x": x[i * BATCH_PER_CORE:(i + 1) * BATCH_PER_CORE].reshape(P, FREE),
         "consts": consts}
        for i in range(N_CORES)
    ]
    res = run_bass_kernel_spmd(nc, in_maps, list(range(N_CORES))).results
    out = np.concatenate(
        [r["out"].reshape(BATCH_PER_CORE, C, 256, 256) for r in res], axis=0
    )
    return out
